# revision 4
# baseline (speedup 1.0000x reference)
"""DEDistMult scoring kernel for Trainium2 (8 NeuronCores, SPMD).

score[b] = sum( concat(e_emb[s_b], t_emb(s_b)) * rel_emb[r_b]
                * concat(e_emb[o_b], t_emb(o_b)) )
with t_emb(e) = sum_c amp_c[e] * sin(frq_c[e] * t_c + phi_c[e]), c in {y,m,d}.

Strategy: data-parallel over B=131072 (16384 per core). All per-entity tables
are concatenated host-side into one [NE, 704] table so each entity needs one
indirect-DMA gather row; rel_emb is gathered separately. Scores are computed
per 128-element tile (batch on partitions), G tiles fused per gather.
"""

import sys

import numpy as np

for _p in ("/opt/trn_rl_repo", "/opt/pypackages"):
    if _p not in sys.path:
        sys.path.append(_p)

B = 131072
NE = 100000
NR = 500
S_DIM = 128
T_DIM = 64
W = S_DIM + 9 * T_DIM  # 704 = e | frq(y,m,d) | phi(y,m,d) | amp(y,m,d)
D = S_DIM + T_DIM  # 192
N_CORES = 8
BS = B // N_CORES  # 16384 per core
P = 128
NT = BS // P  # 128 tiles of 128 elements per core
G = 8  # tiles fused per gather group
NGRP = NT // G

_CACHE = {}


def _build_program():
    import concourse.bass as bass
    import concourse.mybir as mybir
    import concourse.tile as tile
    from concourse import bacc
    from concourse.alu_op_type import AluOpType

    f32 = mybir.dt.float32
    i32 = mybir.dt.int32
    Sin = mybir.ActivationFunctionType.Sin

    nc = bacc.Bacc("TRN2", target_bir_lowering=False, debug=False, num_devices=N_CORES)

    tab_d = nc.dram_tensor("tab", [NE, W], f32, kind="ExternalInput")
    rel_d = nc.dram_tensor("rel", [NR, D], f32, kind="ExternalInput")
    s_d = nc.dram_tensor("s_idx", [P, NT], i32, kind="ExternalInput")
    o_d = nc.dram_tensor("o_idx", [P, NT], i32, kind="ExternalInput")
    r_d = nc.dram_tensor("r_idx", [P, NT], i32, kind="ExternalInput")
    t3_d = nc.dram_tensor("t3", [P, NT * 3], f32, kind="ExternalInput")
    out_d = nc.dram_tensor("out", [P, NT], f32, kind="ExternalOutput")

    with tile.TileContext(nc) as tc:
        with (
            tc.tile_pool(name="const", bufs=1) as const,
            tc.tile_pool(name="gath", bufs=2) as gpool,
            tc.tile_pool(name="work", bufs=2) as wpool,
        ):
            s_sb = const.tile([P, NT], i32)
            o_sb = const.tile([P, NT], i32)
            r_sb = const.tile([P, NT], i32)
            t3_sb = const.tile([P, NT * 3], f32)
            score_sb = const.tile([P, NT], f32)
            nc.sync.dma_start(out=s_sb[:], in_=s_d.ap())
            nc.sync.dma_start(out=o_sb[:], in_=o_d.ap())
            nc.sync.dma_start(out=r_sb[:], in_=r_d.ap())
            nc.sync.dma_start(out=t3_sb[:], in_=t3_d.ap())

            for i in range(NGRP):
                n0 = i * G
                sg = gpool.tile([P, G * W], f32, tag="sg")
                og = gpool.tile([P, G * W], f32, tag="og")
                rg = gpool.tile([P, G * D], f32, tag="rg")
                # HW indirect DMA honors exactly one row index per partition
                # per instruction, so gather the G tiles one by one.
                for g in range(G):
                    n = n0 + g
                    nc.gpsimd.indirect_dma_start(
                        out=sg[:, g * W : (g + 1) * W],
                        out_offset=None,
                        in_=tab_d.ap(),
                        in_offset=bass.IndirectOffsetOnAxis(ap=s_sb[:, n : n + 1], axis=0),
                    )
                    nc.gpsimd.indirect_dma_start(
                        out=og[:, g * W : (g + 1) * W],
                        out_offset=None,
                        in_=tab_d.ap(),
                        in_offset=bass.IndirectOffsetOnAxis(ap=o_sb[:, n : n + 1], axis=0),
                    )
                    nc.gpsimd.indirect_dma_start(
                        out=rg[:, g * D : (g + 1) * D],
                        out_offset=None,
                        in_=rel_d.ap(),
                        in_offset=bass.IndirectOffsetOnAxis(ap=r_sb[:, n : n + 1], axis=0),
                    )

                # t values broadcast: [P, G, 3, 1] -> [P, G, 3, 64]
                tv4 = (
                    t3_sb[:, n0 * 3 : (n0 + G) * 3]
                    .rearrange("p (g c) -> p g c", c=3)
                    .unsqueeze(3)
                    .to_broadcast([P, G, 3, T_DIM])
                )

                tsums = {}
                for ent, gbuf in (("s", sg), ("o", og)):
                    gv = gbuf[:].rearrange("p (g w) -> p g w", g=G)
                    frq4 = gv[:, :, S_DIM : S_DIM + 192].rearrange(
                        "p g (c k) -> p g c k", c=3
                    )
                    args = wpool.tile([P, G * 192], f32, tag=f"args_{ent}")
                    a3 = args[:].rearrange("p (g w) -> p g w", g=G)
                    a4 = args[:].rearrange("p (g c k) -> p g c k", g=G, c=3)
                    # arg = frq * t
                    nc.vector.tensor_tensor(out=a4, in0=frq4, in1=tv4, op=AluOpType.mult)
                    # arg += phi
                    nc.vector.tensor_tensor(
                        out=a3, in0=a3, in1=gv[:, :, 320:512], op=AluOpType.add
                    )
                    # sin
                    nc.scalar.activation(out=args[:], in_=args[:], func=Sin)
                    # *= amp
                    nc.vector.tensor_tensor(
                        out=a3, in0=a3, in1=gv[:, :, 512:704], op=AluOpType.mult
                    )
                    # sum the 3 components -> [P, G, 64]
                    tsum = wpool.tile([P, G * T_DIM], f32, tag=f"tsum_{ent}")
                    tsv = tsum[:].rearrange("p (g k) -> p g k", g=G)
                    nc.vector.tensor_tensor(
                        out=tsv, in0=a3[:, :, 0:64], in1=a3[:, :, 64:128], op=AluOpType.add
                    )
                    nc.vector.tensor_tensor(
                        out=tsv, in0=tsv, in1=a3[:, :, 128:192], op=AluOpType.add
                    )
                    tsums[ent] = tsv

                sgv = sg[:].rearrange("p (g w) -> p g w", g=G)
                ogv = og[:].rearrange("p (g w) -> p g w", g=G)
                prod = wpool.tile([P, G * D], f32, tag="prod")
                p3 = prod[:].rearrange("p (g w) -> p g w", g=G)
                nc.vector.tensor_tensor(
                    out=p3[:, :, 0:S_DIM],
                    in0=sgv[:, :, 0:S_DIM],
                    in1=ogv[:, :, 0:S_DIM],
                    op=AluOpType.mult,
                )
                nc.vector.tensor_tensor(
                    out=p3[:, :, S_DIM:D], in0=tsums["s"], in1=tsums["o"], op=AluOpType.mult
                )
                nc.vector.tensor_tensor(
                    out=p3,
                    in0=p3,
                    in1=rg[:].rearrange("p (g w) -> p g w", g=G),
                    op=AluOpType.mult,
                )
                nc.vector.reduce_sum(
                    out=score_sb[:, n0 : n0 + G].unsqueeze(2),
                    in_=p3,
                    axis=mybir.AxisListType.X,
                )

            nc.sync.dma_start(out=out_d.ap(), in_=score_sb[:])

    nc.compile()
    return nc


def _get_program():
    if "nc" not in _CACHE:
        _CACHE["nc"] = _build_program()
    return _CACHE["nc"]


def _make_in_maps(s, r, o, y, m, d, tab, rel):
    in_maps = []
    for c in range(N_CORES):
        sl = slice(c * BS, (c + 1) * BS)
        t3 = np.ascontiguousarray(
            np.stack([y[sl], m[sl], d[sl]], axis=1).astype(np.float32)
        ).reshape(P, NT * 3)
        in_maps.append(
            {
                "tab": tab,
                "rel": rel,
                "s_idx": np.ascontiguousarray(s[sl].astype(np.int32)).reshape(P, NT),
                "o_idx": np.ascontiguousarray(o[sl].astype(np.int32)).reshape(P, NT),
                "r_idx": np.ascontiguousarray(r[sl].astype(np.int32)).reshape(P, NT),
                "t3": t3,
            }
        )
    return in_maps


def _prep_tables(e_emb, y_frq, m_frq, d_frq, y_phi, m_phi, d_phi, y_amp, m_amp, d_amp, rel_emb):
    tab = np.concatenate(
        [
            np.asarray(e_emb, dtype=np.float32),
            np.asarray(y_frq, dtype=np.float32),
            np.asarray(m_frq, dtype=np.float32),
            np.asarray(d_frq, dtype=np.float32),
            np.asarray(y_phi, dtype=np.float32),
            np.asarray(m_phi, dtype=np.float32),
            np.asarray(d_phi, dtype=np.float32),
            np.asarray(y_amp, dtype=np.float32),
            np.asarray(m_amp, dtype=np.float32),
            np.asarray(d_amp, dtype=np.float32),
        ],
        axis=1,
    )
    rel = np.ascontiguousarray(np.asarray(rel_emb, dtype=np.float32))
    return tab, rel


def kernel(
    s, r, o, y, m, d, e_emb, rel_emb,
    y_frq, m_frq, d_frq, y_phi, m_phi, d_phi, y_amp, m_amp, d_amp,
    _trace=False, _trace_kwargs=None,
):
    from concourse import bass_utils

    nc = _get_program()
    tab, rel = _prep_tables(
        e_emb, y_frq, m_frq, d_frq, y_phi, m_phi, d_phi, y_amp, m_amp, d_amp, rel_emb
    )
    s = np.asarray(s)
    r = np.asarray(r)
    o = np.asarray(o)
    y = np.asarray(y, dtype=np.float32)
    m = np.asarray(m, dtype=np.float32)
    d = np.asarray(d, dtype=np.float32)
    in_maps = _make_in_maps(s, r, o, y, m, d, tab, rel)

    kw = {}
    if _trace:
        kw.update(trace=True, trace_kwargs=_trace_kwargs or {})
    res = bass_utils.run_bass_kernel_spmd(nc, in_maps, list(range(N_CORES)), **kw)
    _CACHE["last_results"] = res
    out = np.concatenate([res.results[c]["out"].reshape(BS) for c in range(N_CORES)])
    return out.astype(np.float32)


# revision 5
# speedup vs baseline: 1.3231x; 1.3231x over previous
"""DEDistMult scoring kernel for Trainium2 (8 NeuronCores, SPMD).

score[b] = sum( concat(e_emb[s_b], t_emb(s_b)) * rel_emb[r_b]
                * concat(e_emb[o_b], t_emb(o_b)) )
with t_emb(e) = sum_c amp_c[e] * sin(frq_c[e] * t_c + phi_c[e]), c in {y,m,d}.

Strategy: data-parallel over B=131072 (16384 per core). The 10 per-entity
tables are concatenated host-side into one [NE, 704] row-table; each core gets
a packed row-shard of just the <=32768 unique entities its batch touches, so
row indices fit int16 and gathers use the multi-row dma_gather primitive
(1024 rows per instruction) instead of one indirect DMA per 128 rows.
Scores are computed per 128-element tile (batch on partitions), G=8 tiles
fused per gather group.
"""

import sys

import numpy as np

for _p in ("/opt/trn_rl_repo", "/opt/pypackages"):
    if _p not in sys.path:
        sys.path.append(_p)

B = 131072
NE = 100000
NR = 500
S_DIM = 128
T_DIM = 64
W = S_DIM + 9 * T_DIM  # 704 = e | frq(y,m,d) | phi(y,m,d) | amp(y,m,d)
D = S_DIM + T_DIM  # 192
N_CORES = 8
BS = B // N_CORES  # 16384 per core
P = 128
NT = BS // P  # 128 tiles of 128 elements per core
G = 8  # tiles fused per gather group
NGRP = NT // G
NU = 2 * BS  # max unique entities per core (= 32768, fits int16)

_CACHE = {}


def _build_program():
    import concourse.bass as bass
    import concourse.mybir as mybir
    import concourse.tile as tile
    from concourse import bacc
    from concourse.alu_op_type import AluOpType

    f32 = mybir.dt.float32
    i16 = mybir.dt.int16
    Sin = mybir.ActivationFunctionType.Sin

    nc = bacc.Bacc("TRN2", target_bir_lowering=False, debug=False, num_devices=N_CORES)

    tab_d = nc.dram_tensor("tab", [NU, W], f32, kind="ExternalInput")
    rel_d = nc.dram_tensor("rel", [NR, D], f32, kind="ExternalInput")
    # int16 indices in dma_gather wrap order: idx i of a group lives at
    # [i % 16, i // 16]; partitions 16h+q replicate partition q.
    s_d = nc.dram_tensor("s_idx", [P, NT * 8], i16, kind="ExternalInput")
    o_d = nc.dram_tensor("o_idx", [P, NT * 8], i16, kind="ExternalInput")
    r_d = nc.dram_tensor("r_idx", [P, NT * 8], i16, kind="ExternalInput")
    t3_d = nc.dram_tensor("t3", [P, NT * 3], f32, kind="ExternalInput")
    out_d = nc.dram_tensor("out", [P, NT], f32, kind="ExternalOutput")

    with tile.TileContext(nc) as tc:
        with (
            tc.tile_pool(name="const", bufs=1) as const,
            tc.tile_pool(name="gath", bufs=2) as gpool,
            tc.tile_pool(name="work", bufs=2) as wpool,
        ):
            s_sb = const.tile([P, NT * 8], i16)
            o_sb = const.tile([P, NT * 8], i16)
            r_sb = const.tile([P, NT * 8], i16)
            t3_sb = const.tile([P, NT * 3], f32)
            score_sb = const.tile([P, NT], f32)
            nc.sync.dma_start(out=s_sb[:], in_=s_d.ap())
            nc.sync.dma_start(out=o_sb[:], in_=o_d.ap())
            nc.sync.dma_start(out=r_sb[:], in_=r_d.ap())
            nc.sync.dma_start(out=t3_sb[:], in_=t3_d.ap())

            for i in range(NGRP):
                n0 = i * G
                ni = G * P  # rows gathered per group per tensor
                jsl = slice(i * ni // 16, (i + 1) * ni // 16)
                sg = gpool.tile([P, G * W], f32, tag="sg")
                og = gpool.tile([P, G * W], f32, tag="og")
                rg = gpool.tile([P, G * D], f32, tag="rg")
                nc.gpsimd.dma_gather(
                    out_ap=sg[:].rearrange("p (g w) -> p g w", g=G),
                    in_ap=tab_d.ap(),
                    idxs_ap=s_sb[:, jsl],
                    num_idxs=ni,
                    num_idxs_reg=ni,
                    elem_size=W,
                )
                nc.gpsimd.dma_gather(
                    out_ap=og[:].rearrange("p (g w) -> p g w", g=G),
                    in_ap=tab_d.ap(),
                    idxs_ap=o_sb[:, jsl],
                    num_idxs=ni,
                    num_idxs_reg=ni,
                    elem_size=W,
                )
                nc.gpsimd.dma_gather(
                    out_ap=rg[:].rearrange("p (g w) -> p g w", g=G),
                    in_ap=rel_d.ap(),
                    idxs_ap=r_sb[:, jsl],
                    num_idxs=ni,
                    num_idxs_reg=ni,
                    elem_size=D,
                )

                # t values broadcast: [P, G, 3, 1] -> [P, G, 3, 64]
                tv4 = (
                    t3_sb[:, n0 * 3 : (n0 + G) * 3]
                    .rearrange("p (g c) -> p g c", c=3)
                    .unsqueeze(3)
                    .to_broadcast([P, G, 3, T_DIM])
                )

                tsums = {}
                for ent, gbuf in (("s", sg), ("o", og)):
                    gv = gbuf[:].rearrange("p (g w) -> p g w", g=G)
                    frq4 = gv[:, :, S_DIM : S_DIM + 192].rearrange(
                        "p g (c k) -> p g c k", c=3
                    )
                    args = wpool.tile([P, G * 192], f32, tag=f"args_{ent}")
                    a3 = args[:].rearrange("p (g w) -> p g w", g=G)
                    a4 = args[:].rearrange("p (g c k) -> p g c k", g=G, c=3)
                    # arg = frq * t
                    nc.vector.tensor_tensor(out=a4, in0=frq4, in1=tv4, op=AluOpType.mult)
                    # arg += phi
                    nc.vector.tensor_tensor(
                        out=a3, in0=a3, in1=gv[:, :, 320:512], op=AluOpType.add
                    )
                    # sin
                    nc.scalar.activation(out=args[:], in_=args[:], func=Sin)
                    # *= amp
                    nc.vector.tensor_tensor(
                        out=a3, in0=a3, in1=gv[:, :, 512:704], op=AluOpType.mult
                    )
                    # sum the 3 components -> [P, G, 64]
                    tsum = wpool.tile([P, G * T_DIM], f32, tag=f"tsum_{ent}")
                    tsv = tsum[:].rearrange("p (g k) -> p g k", g=G)
                    nc.vector.tensor_tensor(
                        out=tsv, in0=a3[:, :, 0:64], in1=a3[:, :, 64:128], op=AluOpType.add
                    )
                    nc.vector.tensor_tensor(
                        out=tsv, in0=tsv, in1=a3[:, :, 128:192], op=AluOpType.add
                    )
                    tsums[ent] = tsv

                sgv = sg[:].rearrange("p (g w) -> p g w", g=G)
                ogv = og[:].rearrange("p (g w) -> p g w", g=G)
                prod = wpool.tile([P, G * D], f32, tag="prod")
                p3 = prod[:].rearrange("p (g w) -> p g w", g=G)
                nc.vector.tensor_tensor(
                    out=p3[:, :, 0:S_DIM],
                    in0=sgv[:, :, 0:S_DIM],
                    in1=ogv[:, :, 0:S_DIM],
                    op=AluOpType.mult,
                )
                nc.vector.tensor_tensor(
                    out=p3[:, :, S_DIM:D], in0=tsums["s"], in1=tsums["o"], op=AluOpType.mult
                )
                nc.vector.tensor_tensor(
                    out=p3,
                    in0=p3,
                    in1=rg[:].rearrange("p (g w) -> p g w", g=G),
                    op=AluOpType.mult,
                )
                nc.vector.reduce_sum(
                    out=score_sb[:, n0 : n0 + G].unsqueeze(2),
                    in_=p3,
                    axis=mybir.AxisListType.X,
                )

            nc.sync.dma_start(out=out_d.ap(), in_=score_sb[:])

    nc.compile()
    return nc


def _get_program():
    if "nc" not in _CACHE:
        _CACHE["nc"] = _build_program()
    return _CACHE["nc"]


def _wrap_idx16(idx_loc):
    """[BS] local indices -> [P, NT*8] int16 in dma_gather wrap order.

    Group i covers batch elements (p, n) for n in [i*G, (i+1)*G); gather
    element index within the group is i_g = g*128 + p, stored at
    [i_g % 16, ni/16*i + i_g // 16]. Globally: [q, 8n + h] = mat[16h+q, n],
    replicated across the 8 partition groups of 16.
    """
    mat = idx_loc.reshape(P, NT).astype(np.int16)
    arr = np.ascontiguousarray(
        mat.reshape(8, 16, NT).transpose(1, 2, 0).reshape(16, NT * 8)
    )
    return np.tile(arr, (8, 1))


def _make_in_maps(s, r, o, y, m, d, tab, rel):
    in_maps = []
    for c in range(N_CORES):
        sl = slice(c * BS, (c + 1) * BS)
        s_c = np.asarray(s[sl], dtype=np.int64)
        o_c = np.asarray(o[sl], dtype=np.int64)
        uniq = np.unique(np.concatenate([s_c, o_c]))
        tab_c = np.zeros((NU, W), dtype=np.float32)
        tab_c[: len(uniq)] = tab[uniq]
        s_loc = np.searchsorted(uniq, s_c)
        o_loc = np.searchsorted(uniq, o_c)
        t3 = np.ascontiguousarray(
            np.stack([y[sl], m[sl], d[sl]], axis=1).astype(np.float32)
        ).reshape(P, NT * 3)
        in_maps.append(
            {
                "tab": tab_c,
                "rel": rel,
                "s_idx": _wrap_idx16(s_loc),
                "o_idx": _wrap_idx16(o_loc),
                "r_idx": _wrap_idx16(np.asarray(r[sl], dtype=np.int64)),
                "t3": t3,
            }
        )
    return in_maps


def _prep_tables(e_emb, y_frq, m_frq, d_frq, y_phi, m_phi, d_phi, y_amp, m_amp, d_amp, rel_emb):
    tab = np.concatenate(
        [
            np.asarray(e_emb, dtype=np.float32),
            np.asarray(y_frq, dtype=np.float32),
            np.asarray(m_frq, dtype=np.float32),
            np.asarray(d_frq, dtype=np.float32),
            np.asarray(y_phi, dtype=np.float32),
            np.asarray(m_phi, dtype=np.float32),
            np.asarray(d_phi, dtype=np.float32),
            np.asarray(y_amp, dtype=np.float32),
            np.asarray(m_amp, dtype=np.float32),
            np.asarray(d_amp, dtype=np.float32),
        ],
        axis=1,
    )
    rel = np.ascontiguousarray(np.asarray(rel_emb, dtype=np.float32))
    return tab, rel


def kernel(
    s, r, o, y, m, d, e_emb, rel_emb,
    y_frq, m_frq, d_frq, y_phi, m_phi, d_phi, y_amp, m_amp, d_amp,
    _trace=False, _trace_kwargs=None,
):
    from concourse import bass_utils

    nc = _get_program()
    tab, rel = _prep_tables(
        e_emb, y_frq, m_frq, d_frq, y_phi, m_phi, d_phi, y_amp, m_amp, d_amp, rel_emb
    )
    s = np.asarray(s)
    r = np.asarray(r)
    o = np.asarray(o)
    y = np.asarray(y, dtype=np.float32)
    m = np.asarray(m, dtype=np.float32)
    d = np.asarray(d, dtype=np.float32)
    in_maps = _make_in_maps(s, r, o, y, m, d, tab, rel)

    kw = {}
    if _trace:
        kw.update(trace=True, trace_kwargs=_trace_kwargs or {})
    res = bass_utils.run_bass_kernel_spmd(nc, in_maps, list(range(N_CORES)), **kw)
    _CACHE["last_results"] = res
    out = np.concatenate([res.results[c]["out"].reshape(BS) for c in range(N_CORES)])
    return out.astype(np.float32)


# revision 8
# speedup vs baseline: 1.6725x; 1.2641x over previous
"""DEDistMult scoring kernel for Trainium2 (8 NeuronCores, SPMD).

score[b] = sum( concat(e_emb[s_b], t_emb(s_b)) * rel_emb[r_b]
                * concat(e_emb[o_b], t_emb(o_b)) )
with t_emb(e) = sum_c amp_c[e] * sin(frq_c[e] * t_c + phi_c[e]), c in {y,m,d}.

Strategy: data-parallel over B=131072 (16384 per core).

- Entity tables (e_emb + 9 time tables) are concatenated host-side into one
  bf16 row-table; each core gets a packed row-shard of just the <=32768
  unique entities its batch touches, so row indices fit int16 and gathers
  use the multi-row dma_gather primitive (1024 rows / instruction). The Q7
  descriptor-generation rate (~9ns/row) is the kernel's critical path, so
  only the two entity gathers (s, o) ride it.
- rel_emb (500 rows) stays SBUF-resident; per-element rows are produced on
  the idle TensorEngine as onehot(r) @ rel chunks, with the onehot built by
  a DVE is_equal against a replicated-r layout prepared host-side.
- Compute is bf16 (DVE tensor_tensor gets its 2x perf mode), scores reduce
  to fp32.
"""

import sys

import numpy as np

for _p in ("/opt/trn_rl_repo", "/opt/pypackages"):
    if _p not in sys.path:
        sys.path.append(_p)

B = 131072
NE = 100000
NR = 500
S_DIM = 128
T_DIM = 64
WD = S_DIM + 9 * T_DIM  # 704 data cols: e | frq(y,m,d) | phi(y,m,d) | amp(y,m,d)
W = 768  # padded row length so bf16 rows are 1536B (multiple of 256)
D = S_DIM + T_DIM  # 192
NRC = 4  # rel vocab chunks of 128 (500 -> 512)
N_CORES = 8
BS = B // N_CORES  # 16384 per core
P = 128
NT = BS // P  # 128 tiles of 128 elements per core
G = 8  # tiles fused per gather group (1024 rows = dma_gather max)
NGRP = NT // G
NU = 2 * BS  # max unique entities per core (= 32768, fits int16)

_CACHE = {}


def _build_program():
    import concourse.bass as bass
    import concourse.mybir as mybir
    import concourse.tile as tile
    from concourse import bacc
    from concourse.alu_op_type import AluOpType

    f32 = mybir.dt.float32
    bf16 = mybir.dt.bfloat16
    i16 = mybir.dt.int16
    Sin = mybir.ActivationFunctionType.Sin

    nc = bacc.Bacc("TRN2", target_bir_lowering=False, debug=False, num_devices=N_CORES)

    tab_d = nc.dram_tensor("tab", [NU, W], bf16, kind="ExternalInput")
    rel_d = nc.dram_tensor("rel", [P, NRC * D], bf16, kind="ExternalInput")
    s_d = nc.dram_tensor("s_idx", [P, NT * 8], i16, kind="ExternalInput")
    o_d = nc.dram_tensor("o_idx", [P, NT * 8], i16, kind="ExternalInput")
    rrep_d = nc.dram_tensor("rrep", [P, NT * P], f32, kind="ExternalInput")
    iota_d = nc.dram_tensor("iota4", [P, NRC], f32, kind="ExternalInput")
    t3_d = nc.dram_tensor("t3", [P, NT * 3], f32, kind="ExternalInput")
    out_d = nc.dram_tensor("out", [P, NT], f32, kind="ExternalOutput")

    with tile.TileContext(nc) as tc:
        with (
            tc.tile_pool(name="const", bufs=1) as const,
            tc.tile_pool(name="gath", bufs=3) as gpool,
            tc.tile_pool(name="work", bufs=2) as wpool,
            tc.tile_pool(name="psum", bufs=1, space="PSUM") as ppool,
        ):
            s_sb = const.tile([P, NT * 8], i16)
            o_sb = const.tile([P, NT * 8], i16)
            rel_sb = const.tile([P, NRC * D], bf16)
            iota_sb = const.tile([P, NRC], f32)
            t3_sb = const.tile([P, NT * 3], f32)
            score_sb = const.tile([P, NT], f32)
            nc.sync.dma_start(out=s_sb[:], in_=s_d.ap())
            nc.sync.dma_start(out=o_sb[:], in_=o_d.ap())
            nc.sync.dma_start(out=rel_sb[:], in_=rel_d.ap())
            nc.sync.dma_start(out=iota_sb[:], in_=iota_d.ap())
            nc.sync.dma_start(out=t3_sb[:], in_=t3_d.ap())

            for i in range(NGRP):
                n0 = i * G
                ni = G * P  # rows gathered per group per entity
                jsl = slice(i * ni // 16, (i + 1) * ni // 16)
                sg = gpool.tile([P, G * W], bf16, tag="sg")
                og = gpool.tile([P, G * W], bf16, tag="og")
                nc.gpsimd.dma_gather(
                    out_ap=sg[:].rearrange("p (g w) -> p g w", g=G),
                    in_ap=tab_d.ap(),
                    idxs_ap=s_sb[:, jsl],
                    num_idxs=ni,
                    num_idxs_reg=ni,
                    elem_size=W,
                )
                nc.gpsimd.dma_gather(
                    out_ap=og[:].rearrange("p (g w) -> p g w", g=G),
                    in_ap=tab_d.ap(),
                    idxs_ap=o_sb[:, jsl],
                    num_idxs=ni,
                    num_idxs_reg=ni,
                    elem_size=W,
                )

                # rel rows via onehot(r) @ rel on the TensorEngine
                rr = wpool.tile([P, G * P], f32, tag="rr")
                nc.sync.dma_start(out=rr[:], in_=rrep_d.ap()[:, n0 * P : (n0 + G) * P])
                # PSUM matmul outputs must not cross a 512-f32 bank boundary:
                # give each tile a 256-f32 slot (192 used).
                relg_ps = ppool.tile([P, G * 256], f32, tag="relg")
                ohs = []
                for c in range(NRC):
                    oh = wpool.tile([P, G * P], bf16, tag=f"oh{c}")
                    nc.vector.tensor_scalar(
                        out=oh[:],
                        in0=rr[:],
                        scalar1=iota_sb[:, c : c + 1],
                        scalar2=None,
                        op0=AluOpType.is_equal,
                    )
                    ohs.append(oh)
                for g in range(G):
                    for c in range(NRC):
                        nc.tensor.matmul(
                            relg_ps[:, g * 256 : g * 256 + D],
                            ohs[c][:, g * P : (g + 1) * P],
                            rel_sb[:, c * D : (c + 1) * D],
                            start=(c == 0),
                            stop=(c == NRC - 1),
                        )

                # t values broadcast: [P, G, 3, 1] -> [P, G, 3, 64]
                tv4 = (
                    t3_sb[:, n0 * 3 : (n0 + G) * 3]
                    .rearrange("p (g c) -> p g c", c=3)
                    .unsqueeze(3)
                    .to_broadcast([P, G, 3, T_DIM])
                )

                tsums = {}
                for ent, gbuf in (("s", sg), ("o", og)):
                    gv = gbuf[:].rearrange("p (g w) -> p g w", g=G)
                    frq4 = gv[:, :, S_DIM : S_DIM + 192].rearrange(
                        "p g (c k) -> p g c k", c=3
                    )
                    args = wpool.tile([P, G * 192], bf16, tag=f"args_{ent}")
                    a3 = args[:].rearrange("p (g w) -> p g w", g=G)
                    a4 = args[:].rearrange("p (g c k) -> p g c k", g=G, c=3)
                    # arg = frq * t
                    nc.vector.tensor_tensor(out=a4, in0=frq4, in1=tv4, op=AluOpType.mult)
                    # arg += phi
                    nc.vector.tensor_tensor(
                        out=a3, in0=a3, in1=gv[:, :, 320:512], op=AluOpType.add
                    )
                    # sin
                    nc.scalar.activation(out=args[:], in_=args[:], func=Sin)
                    # *= amp
                    nc.vector.tensor_tensor(
                        out=a3, in0=a3, in1=gv[:, :, 512:704], op=AluOpType.mult
                    )
                    # sum the 3 components -> [P, G, 64]
                    tsum = wpool.tile([P, G * T_DIM], bf16, tag=f"tsum_{ent}")
                    tsv = tsum[:].rearrange("p (g k) -> p g k", g=G)
                    nc.vector.tensor_tensor(
                        out=tsv, in0=a3[:, :, 0:64], in1=a3[:, :, 64:128], op=AluOpType.add
                    )
                    nc.vector.tensor_tensor(
                        out=tsv, in0=tsv, in1=a3[:, :, 128:192], op=AluOpType.add
                    )
                    tsums[ent] = tsv

                sgv = sg[:].rearrange("p (g w) -> p g w", g=G)
                ogv = og[:].rearrange("p (g w) -> p g w", g=G)
                prod = wpool.tile([P, G * D], bf16, tag="prod")
                p3 = prod[:].rearrange("p (g w) -> p g w", g=G)
                nc.vector.tensor_tensor(
                    out=p3[:, :, 0:S_DIM],
                    in0=sgv[:, :, 0:S_DIM],
                    in1=ogv[:, :, 0:S_DIM],
                    op=AluOpType.mult,
                )
                nc.vector.tensor_tensor(
                    out=p3[:, :, S_DIM:D], in0=tsums["s"], in1=tsums["o"], op=AluOpType.mult
                )
                nc.vector.tensor_tensor(
                    out=p3,
                    in0=p3,
                    in1=relg_ps[:].rearrange("p (g w) -> p g w", g=G)[:, :, 0:D],
                    op=AluOpType.mult,
                )
                nc.vector.reduce_sum(
                    out=score_sb[:, n0 : n0 + G].unsqueeze(2),
                    in_=p3,
                    axis=mybir.AxisListType.X,
                )

            nc.sync.dma_start(out=out_d.ap(), in_=score_sb[:])

    nc.compile()
    return nc


def _get_program():
    if "nc" not in _CACHE:
        _CACHE["nc"] = _build_program()
    return _CACHE["nc"]


def _wrap_idx16(idx_loc):
    """[BS] local indices -> [P, NT*8] int16 in dma_gather wrap order.

    Gather element i of group grp (i = g*128 + p for dest [p, g]) reads its
    index from [i % 16, 64*grp + i // 16]. Globally: [q, 8n + h] =
    mat[16h+q, n], replicated across the 8 partition groups of 16.
    """
    mat = idx_loc.reshape(P, NT).astype(np.int16)
    arr = np.ascontiguousarray(
        mat.reshape(8, 16, NT).transpose(1, 2, 0).reshape(16, NT * 8)
    )
    return np.tile(arr, (8, 1))


def _make_in_maps(s, r, o, y, m, d, tab, rel_pack, iota4):
    in_maps = []
    for c in range(N_CORES):
        sl = slice(c * BS, (c + 1) * BS)
        s_c = np.asarray(s[sl], dtype=np.int64)
        o_c = np.asarray(o[sl], dtype=np.int64)
        r_c = np.asarray(r[sl], dtype=np.int64)
        uniq = np.unique(np.concatenate([s_c, o_c]))
        tab_c = np.zeros((NU, W), dtype=tab.dtype)
        tab_c[: len(uniq), :WD] = tab[uniq]
        s_loc = np.searchsorted(uniq, s_c)
        o_loc = np.searchsorted(uniq, o_c)
        # replicated-r layout: [q, n*128 + p] = r[(p, n)] for every partition q
        rrow = np.ascontiguousarray(
            r_c.reshape(P, NT).T.reshape(1, NT * P).astype(np.float32)
        )
        rrep = np.broadcast_to(rrow, (P, NT * P))
        t3 = np.ascontiguousarray(
            np.stack([y[sl], m[sl], d[sl]], axis=1).astype(np.float32)
        ).reshape(P, NT * 3)
        in_maps.append(
            {
                "tab": tab_c,
                "rel": rel_pack,
                "s_idx": _wrap_idx16(s_loc),
                "o_idx": _wrap_idx16(o_loc),
                "rrep": rrep,
                "iota4": iota4,
                "t3": t3,
            }
        )
    return in_maps


def _prep_tables(e_emb, y_frq, m_frq, d_frq, y_phi, m_phi, d_phi, y_amp, m_amp, d_amp, rel_emb):
    import ml_dtypes

    bf = ml_dtypes.bfloat16
    tab = np.concatenate(
        [
            np.asarray(e_emb, dtype=np.float32),
            np.asarray(y_frq, dtype=np.float32),
            np.asarray(m_frq, dtype=np.float32),
            np.asarray(d_frq, dtype=np.float32),
            np.asarray(y_phi, dtype=np.float32),
            np.asarray(m_phi, dtype=np.float32),
            np.asarray(d_phi, dtype=np.float32),
            np.asarray(y_amp, dtype=np.float32),
            np.asarray(m_amp, dtype=np.float32),
            np.asarray(d_amp, dtype=np.float32),
        ],
        axis=1,
    ).astype(bf)
    # rel packed into vocab chunks of 128: [v, c*D + k] = rel[c*128 + v, k]
    rel_f = np.zeros((NRC * P, D), dtype=np.float32)
    rel_f[:NR] = np.asarray(rel_emb, dtype=np.float32)
    rel_pack = np.ascontiguousarray(
        rel_f.reshape(NRC, P, D).transpose(1, 0, 2).reshape(P, NRC * D)
    ).astype(bf)
    iota4 = (
        np.arange(P, dtype=np.float32)[:, None] + 128.0 * np.arange(NRC, dtype=np.float32)
    )
    return tab, rel_pack, np.ascontiguousarray(iota4)


def kernel(
    s, r, o, y, m, d, e_emb, rel_emb,
    y_frq, m_frq, d_frq, y_phi, m_phi, d_phi, y_amp, m_amp, d_amp,
    _trace=False, _trace_kwargs=None,
):
    from concourse import bass_utils

    nc = _get_program()
    tab, rel_pack, iota4 = _prep_tables(
        e_emb, y_frq, m_frq, d_frq, y_phi, m_phi, d_phi, y_amp, m_amp, d_amp, rel_emb
    )
    s = np.asarray(s)
    r = np.asarray(r)
    o = np.asarray(o)
    y = np.asarray(y, dtype=np.float32)
    m = np.asarray(m, dtype=np.float32)
    d = np.asarray(d, dtype=np.float32)
    in_maps = _make_in_maps(s, r, o, y, m, d, tab, rel_pack, iota4)

    kw = {}
    if _trace:
        kw.update(trace=True, trace_kwargs=_trace_kwargs or {})
    res = bass_utils.run_bass_kernel_spmd(nc, in_maps, list(range(N_CORES)), **kw)
    _CACHE["last_results"] = res
    out = np.concatenate([res.results[c]["out"].reshape(BS) for c in range(N_CORES)])
    return out.astype(np.float32)
